# revision 3
# baseline (speedup 1.0000x reference)
"""Peephole-LSTM cell fused kernel for 8 Trainium2 NeuronCores.

Math (per reference):
    pre = X_t @ W + c_prev @ U + b          # W/U/b are the 4 gates concat'd
    f, i, o, c_hat = split(pre);  f,i,o = sigmoid;  c_hat = tanh
    c_t = f * c_prev + i * c_hat
    h_t = o * tanh(c_t)

Sharding: data-parallel over the batch dim (16384 -> 8 x 2048), weights
replicated, no cross-device communication.

Per-core device plan (B_loc=2048, D=512, 4H=2048):
  - Host pre-transposes X and c_prev, casts to fp16, and splits every
    operand into exactly the granularity the device consumes (free), so
    the device does nothing but 512 [128k x 128m x 512n] fp16 matmuls
    plus the elementwise gate chain.
  - CRITICAL scheduling rule: PSUM accumulation groups are NEVER
    interleaved.  A k-outer pattern that round-robins matmuls across
    several open PSUM banks drops the PE clock from ~2.37 GHz to
    ~1.98 GHz for the REST OF THE KERNEL (259 ns vs 215.8 ns per
    matmul, +23 us total).  Every chain here runs start..stop to
    completion before the next group opens.
  - Operands stream over 2 serial DMA lanes (HBM is the binding
    constraint at ~358 GB/s per core; 2 lanes split it) in exact
    consumption order: W gate 3 k-pieces + X first, so the first real
    chain is complete by ~7 us; C/U/cn follow; late batch-halves last.
    Junk matmuls bridge boot -> first operands and ramp the PE clock.
  - Phase A runs X@W for the first N_STAGED tiles gate-major as
    sequential 4-chains staged to SBUF fp16 (PX); phase B interleaves
    staged tiles (C@U 4-chain + PX add) with fused tiles (8-chains) so
    the staged tiles' extra vector adds spread out.
  - Elementwise: scalar does sigmoid/tanh straight from PSUM; the
    c_t/h chain is split across VectorE and GpSimd; c_prev re-loads as
    fp16 [batch, H] tiles (error ~2e-3 absolute in c_t, well inside
    the tolerance).  The final tile pipelines its i-path in 256-col
    chunks so tanh(c_t) is ready before the o-chain ends; only
    sigmoid(o) + multiply + one store trail the last matmul (~2 us).
"""

import sys

if "/opt/trn_rl_repo" not in sys.path:
    sys.path.insert(0, "/opt/trn_rl_repo")

import numpy as np

import concourse.bacc as bacc
import concourse.mybir as mybir
import concourse.tile as tile
from concourse import bass_utils

N_CORES = 8
B, D, H = 16384, 512, 512
BL = B // N_CORES          # 2048 rows per core
G4 = 4 * H                 # 2048, the concatenated gate dim
KT = D // 128              # 4 k-tiles
BT = BL // 128             # 16 batch tiles per core
WARMUP_MMS = 8             # junk matmuls to lift the HAM clock gate
N_STAGED = 4               # batch tiles that run X@W-only while C/U load
BANK_ORDER = (3, 1, 0, 2)  # c_hat, i, f, o — deep chain first
LAST_ORDER = (3, 0, 1, 2)  # final tile: f early so c_t closes pre-end
# staged tiles interleaved with fused ones to spread their vector adds
TILE_ORDER = (0, 4, 1, 5, 2, 6, 3, 7, 8, 9, 10, 11, 12, 13, 14, 15)
# cn (c_prev fp16, natural layout) four-tile blocks follow TILE_ORDER
CN_BLOCKS = ((0, 4, 1, 5), (2, 6, 3, 7), (8, 9, 10, 11), (12, 13, 14, 15))

_cached = {}


def _build(has_bias: bool):
    key = has_bias
    if key in _cached:
        return _cached[key]

    f32 = mybir.dt.float32
    f16 = mybir.dt.float16
    AF = mybir.ActivationFunctionType
    Alu = mybir.AluOpType

    nc = bacc.Bacc("TRN2", target_bir_lowering=False, debug=False,
                   enable_asserts=False, enable_partition_id=False)
    # X^T / C^T per (k, batch-half): [128, 1024] fp16
    xa = [nc.dram_tensor(f"xa{k}", [128, 1024], f16, kind="ExternalInput")
          for k in range(KT)]
    xb = [nc.dram_tensor(f"xb{k}", [128, 1024], f16, kind="ExternalInput")
          for k in range(KT)]
    ca = [nc.dram_tensor(f"ca{k}", [128, 1024], f16, kind="ExternalInput")
          for k in range(KT)]
    cb = [nc.dram_tensor(f"cb{k}", [128, 1024], f16, kind="ExternalInput")
          for k in range(KT)]
    # W / U per (gate, k): [128, 512] fp16
    wd = {(jc, k): nc.dram_tensor(f"w{jc}_{k}", [128, 512], f16,
                                  kind="ExternalInput")
          for jc in range(4) for k in range(KT)}
    ud = {(jc, k): nc.dram_tensor(f"u{jc}_{k}", [128, 512], f16,
                                  kind="ExternalInput")
          for jc in range(4) for k in range(KT)}
    # c_prev natural-layout fp16, 4 tiles per block in TILE_ORDER
    cnb_d = [nc.dram_tensor(f"cnb{g}", [128, 4 * H], f16,
                            kind="ExternalInput")
             for g in range(4)]
    if has_bias:
        bias_bc = nc.dram_tensor("bias_bc", [128, G4], f32,
                                 kind="ExternalInput")
    h_out = nc.dram_tensor("h_out", [BL, H], f32, kind="ExternalOutput")

    with tile.TileContext(nc) as tc:
        with (
            tc.tile_pool(name="const", bufs=1) as const,
            tc.tile_pool(name="px", bufs=1) as px_p,
            tc.tile_pool(name="psum", bufs=8, space="PSUM") as psum,
            tc.tile_pool(name="gates", bufs=8) as gate_p,
            tc.tile_pool(name="tmp1", bufs=4) as tmp1_p,
            tc.tile_pool(name="tmp2", bufs=4) as tmp2_p,
            tc.tile_pool(name="hsb", bufs=3) as h_p,
        ):
            # PE warm-up: burn the boot->operand window on junk matmuls
            # so the HAM clock gate sees sustained activity.  gpsimd
            # memset: its sequencer preamble finishes earliest, so the
            # first junk matmul issues ~2 us sooner than via vector.
            junk = const.tile([128, 512], f16, tag="junk", name="junk")
            nc.gpsimd.memset(junk[:], 0.0)
            # Pre-create the PSUM tiles of the first 8 phase-A chains
            # and aim the junk matmuls at them: real chains begin
            # start=True (bank reset), and targeting later-read tiles
            # keeps bacc's DCE from dropping the junk.
            psA = {}
            for jc in (3, 2):
                for bt in range(N_STAGED):
                    psA[(jc, bt)] = psum.tile([128, 512], f32, tag="ps",
                                              name=f"psA{jc}_{bt}")
            warm_tgts = list(psA.values())
            for i in range(WARMUP_MMS):
                nc.tensor.matmul(
                    warm_tgts[i % 8][:], junk[:, 0:128], junk[:],
                    start=True, stop=True,
                )

            # SBUF operand tiles
            def sb(name, cols, dt=f16):
                return const.tile([128, cols], dt, tag=name, name=name)

            XA = [sb(f"XA{k}", 1024) for k in range(KT)]
            XB = [sb(f"XB{k}", 1024) for k in range(KT)]
            CA = [sb(f"CA{k}", 1024) for k in range(KT)]
            CB = [sb(f"CB{k}", 1024) for k in range(KT)]
            Wt = {key: sb(f"W{key[0]}_{key[1]}", 512) for key in wd}
            Ut = {key: sb(f"U{key[0]}_{key[1]}", 512) for key in ud}
            CNB = [sb(f"CNB{g}", 4 * H) for g in range(4)]

            # Two serial DMA lanes in exact consumption order.  Each
            # item gates on the item TWO back in its lane (2 transfers
            # in flight hide semaphore latency, order preserved).
            laneA = []  # weights + cn
            for jc in (3, 2, 1, 0):
                for k in range(KT):
                    laneA.append((Wt[(jc, k)], wd[(jc, k)]))
            for jc in (3, 1):
                for k in range(KT):
                    laneA.append((Ut[(jc, k)], ud[(jc, k)]))
            laneA.append((CNB[0], cnb_d[0]))
            for jc in (0, 2):
                for k in range(KT):
                    laneA.append((Ut[(jc, k)], ud[(jc, k)]))
            laneA.append((CNB[1], cnb_d[1]))
            laneA.append((CNB[2], cnb_d[2]))
            laneA.append((CNB[3], cnb_d[3]))
            laneB = []  # activations
            for k in range(KT):
                laneB.append((XA[k], xa[k]))
            for k in range(KT):
                laneB.append((CA[k], ca[k]))
            for k in range(KT):
                laneB.append((XB[k], xb[k]))
            for k in range(KT):
                laneB.append((CB[k], cb[k]))

            for li, lane in enumerate((laneA, laneB)):
                for wave, (dst, src) in enumerate(lane):
                    if wave > 1:
                        prev = lane[wave - 2][0]
                        nc.vector.tensor_copy(dst[0:1, 0:1], prev[0:1, 0:1])
                        eng = nc.sync
                    else:
                        eng = nc.scalar if li else nc.sync
                    eng.dma_start(out=dst[:], in_=src.ap())
            if has_bias:
                bias_sb = const.tile([128, G4], f32, tag="bias")
                nc.sync.dma_start(out=bias_sb[:], in_=bias_bc.ap())

            def lhs(k, bt):
                q, r = divmod(bt, 8)
                T = (XA, XB)[q]
                return T[k][:, r * 128:(r + 1) * 128]

            def lhc(k, bt):
                q, r = divmod(bt, 8)
                T = (CA, CB)[q]
                return T[k][:, r * 128:(r + 1) * 128]

            # Phase A: X@W for staged tiles, gate-major sequential
            # 4-chains (matches W arrival: all jc=3 pieces, then jc=2,
            # ...), staged to SBUF fp16.
            PX = [px_p.tile([128, G4], f16, tag=f"px{bt}", name=f"px{bt}")
                  for bt in range(N_STAGED)]
            for jc in (3, 2, 1, 0):
                for bt in range(N_STAGED):
                    if (jc, bt) in psA:
                        ps = psA[(jc, bt)]
                    else:
                        ps = psum.tile([128, 512], f32, tag="ps",
                                       name=f"psA{jc}_{bt}")
                    for k in range(KT):
                        nc.tensor.matmul(
                            ps[:], lhs(k, bt), Wt[(jc, k)][:],
                            start=(k == 0), stop=(k == KT - 1),
                        )
                    nc.scalar.activation(
                        PX[bt][:, jc * 512:(jc + 1) * 512], ps[:], AF.Copy
                    )

            def cn_ap(bt):
                for g, blk in enumerate(CN_BLOCKS):
                    if bt in blk:
                        j = blk.index(bt)
                        return CNB[g][:, j * H:(j + 1) * H]
                raise AssertionError

            # Phase B: per tile, fill the four gate banks (sequential
            # chains) and run the eviction/elementwise chain.
            for bt in TILE_ORDER:
                bsl = slice(bt * 128, (bt + 1) * 128)
                last = bt == BT - 1
                cn = cn_ap(bt)
                staged = bt < N_STAGED
                order = LAST_ORDER if last else BANK_ORDER
                gates = {}
                for jc in order:
                    jsl = slice(jc * 512, (jc + 1) * 512)
                    ps = psum.tile([128, 512], f32, tag="ps",
                                   name=f"psB{bt}_{jc}")
                    if staged:
                        for k in range(KT):
                            nc.tensor.matmul(
                                ps[:], lhc(k, bt), Ut[(jc, k)][:],
                                start=(k == 0), stop=(k == KT - 1),
                            )
                        nc.vector.tensor_tensor(
                            ps[:], ps[:], PX[bt][:, jsl], Alu.add
                        )
                    else:
                        for k in range(KT):
                            nc.tensor.matmul(
                                ps[:], lhs(k, bt), Wt[(jc, k)][:],
                                start=(k == 0), stop=False,
                            )
                        for k in range(KT):
                            nc.tensor.matmul(
                                ps[:], lhc(k, bt), Ut[(jc, k)][:],
                                start=False, stop=(k == KT - 1),
                            )
                    if has_bias:
                        nc.vector.tensor_tensor(
                            ps[:], ps[:], bias_sb[:, jsl], Alu.add
                        )
                    g = gate_p.tile([128, 512], f32, tag="g",
                                    name=f"g{bt}_{jc}")
                    if last and jc in (1, 2):
                        # final tile: halve the eviction latency of the
                        # i and o gates so the closing chain pipelines.
                        for hs in range(2):
                            csl = slice(hs * 256, (hs + 1) * 256)
                            nc.scalar.activation(
                                g[:, csl], ps[:, csl], AF.Sigmoid
                            )
                    else:
                        nc.scalar.activation(
                            g[:], ps[:], AF.Tanh if jc == 3 else AF.Sigmoid
                        )
                    gates[jc] = g
                    # Chain steps as their inputs become ready.
                    if jc == 0:
                        t1 = tmp1_p.tile([128, H], f32, tag="t1",
                                         name=f"t1_{bt}")
                        nc.gpsimd.tensor_tensor(
                            t1[:], gates[0][:], cn, Alu.mult
                        )
                    elif jc == 1:
                        t2 = tmp2_p.tile([128, H], f32, tag="t2",
                                         name=f"t2_{bt}")
                        if last:
                            # 256-col chunks so tanh(c_t) lands before
                            # the o-chain finishes.
                            tct = tmp2_p.tile([128, H], f32, tag="tct",
                                              name=f"tct{bt}")
                            for hs in range(2):
                                csl = slice(hs * 256, (hs + 1) * 256)
                                nc.vector.tensor_tensor(
                                    t2[:, csl], gates[1][:, csl],
                                    gates[3][:, csl], Alu.mult,
                                )
                                nc.vector.tensor_tensor(
                                    t1[:, csl], t1[:, csl], t2[:, csl],
                                    Alu.add,
                                )
                                nc.scalar.activation(
                                    tct[:, csl], t1[:, csl], AF.Tanh
                                )
                        else:
                            nc.vector.tensor_tensor(
                                t2[:], gates[1][:], gates[3][:], Alu.mult
                            )
                    elif jc == 2:
                        if not last:
                            nc.vector.tensor_tensor(t1[:], t1[:], t2[:],
                                                    Alu.add)
                            tct = tmp2_p.tile([128, H], f32, tag="tct",
                                              name=f"tct{bt}")
                            nc.scalar.activation(tct[:], t1[:], AF.Tanh)
                        hsb = h_p.tile([128, H], f32, tag="h", name=f"h{bt}")
                        if last:
                            for hs in range(2):
                                csl = slice(hs * 256, (hs + 1) * 256)
                                nc.vector.tensor_tensor(
                                    hsb[:, csl], gates[2][:, csl],
                                    tct[:, csl], Alu.mult,
                                )
                            nc.sync.dma_start(out=h_out.ap()[bsl, :],
                                              in_=hsb[:])
                        else:
                            nc.gpsimd.tensor_tensor(
                                hsb[:], gates[2][:], tct[:], Alu.mult
                            )
                            nc.sync.dma_start(out=h_out.ap()[bsl, :],
                                              in_=hsb[:])

    nc.compile()
    _cached[key] = nc
    return nc


def _prep(X_t, c_prev, W_f, W_i, W_o, W_c, U_f, U_i, U_o, U_c, b_f, b_i, b_o, b_c):
    """Host-side (free) preprocessing: concat, cast, transpose, shard."""
    f16 = np.float16
    W = np.concatenate([W_f, W_i, W_o, W_c], axis=1).astype(f16)
    U = np.concatenate([U_f, U_i, U_o, U_c], axis=1).astype(f16)
    b = np.concatenate([b_f, b_i, b_o, b_c], axis=0).astype(np.float32)
    has_bias = bool(np.any(b != 0.0))

    X16 = np.asarray(X_t).astype(f16)
    C16 = np.asarray(c_prev).astype(f16)

    wp = {}
    for jc in range(4):
        for k in range(KT):
            wp[f"w{jc}_{k}"] = np.ascontiguousarray(
                W[k * 128:(k + 1) * 128, jc * 512:(jc + 1) * 512]
            )
            wp[f"u{jc}_{k}"] = np.ascontiguousarray(
                U[k * 128:(k + 1) * 128, jc * 512:(jc + 1) * 512]
            )

    in_maps = []
    for i in range(N_CORES):
        sl = slice(i * BL, (i + 1) * BL)
        XT = X16[sl].T  # [D, BL]
        CT = C16[sl].T
        Cn = C16[sl]    # [BL, D] natural
        m = dict(wp)
        for k in range(KT):
            ks = slice(k * 128, (k + 1) * 128)
            m[f"xa{k}"] = np.ascontiguousarray(XT[ks, 0:1024])
            m[f"xb{k}"] = np.ascontiguousarray(XT[ks, 1024:2048])
            m[f"ca{k}"] = np.ascontiguousarray(CT[ks, 0:1024])
            m[f"cb{k}"] = np.ascontiguousarray(CT[ks, 1024:2048])
        for g, blk in enumerate(CN_BLOCKS):
            m[f"cnb{g}"] = np.ascontiguousarray(
                np.concatenate([Cn[bt * 128:(bt + 1) * 128, :] for bt in blk],
                               axis=1)
            )
        if has_bias:
            m["bias_bc"] = np.ascontiguousarray(
                np.broadcast_to(b[None, :], (128, G4))
            )
        in_maps.append(m)
    return in_maps, has_bias


def kernel(**inputs):
    in_maps, has_bias = _prep(**inputs)
    nc = _build(has_bias)
    last_err = None
    for _ in range(3):
        try:
            res = bass_utils.run_bass_kernel_spmd(
                nc, in_maps, core_ids=list(range(N_CORES))
            )
            break
        except Exception as e:  # intermittent device wedge: retry
            last_err = e
            import time
            time.sleep(5)
    else:
        raise last_err
    return np.concatenate([res.results[i]["h_out"] for i in range(N_CORES)],
                          axis=0)


# revision 4
# speedup vs baseline: 1.2143x; 1.2143x over previous
"""Peephole-LSTM cell fused kernel for 8 Trainium2 NeuronCores.

Math (per reference):
    pre = X_t @ W + c_prev @ U + b          # W/U/b are the 4 gates concat'd
    f, i, o, c_hat = split(pre);  f,i,o = sigmoid;  c_hat = tanh
    c_t = f * c_prev + i * c_hat
    h_t = o * tanh(c_t)

Sharding: data-parallel over the batch dim (16384 -> 8 x 2048), weights
replicated, no cross-device communication.

Per-core device plan (B_loc=2048, D=512, 4H=2048):
  - Host pre-transposes X and c_prev, casts to fp16, and splits every
    operand into exactly the granularity the device consumes (free), so
    the device does nothing but 512 [128k x 128m x 512n] fp16 matmuls
    plus the elementwise gate chain.
  - CRITICAL scheduling rule: PSUM accumulation groups are NEVER
    interleaved.  A k-outer pattern that round-robins matmuls across
    several open PSUM banks drops the PE clock from ~2.37 GHz to
    ~1.98 GHz for the REST OF THE KERNEL (259 ns vs 215.8 ns per
    matmul, +23 us total).  Every chain here runs start..stop to
    completion before the next group opens.
  - Operands stream over 2 serial DMA lanes (HBM is the binding
    constraint at ~358 GB/s per core; 2 lanes split it) in exact
    consumption order: W gate 3 k-pieces + X first, so the first real
    chain is complete by ~7 us; C/U/cn follow; late batch-halves last.
    Junk matmuls bridge boot -> first operands and ramp the PE clock.
  - Phase A runs X@W for the first N_STAGED tiles gate-major as
    sequential 4-chains staged to SBUF fp16 (PX); phase B interleaves
    staged tiles (C@U 4-chain + PX add) with fused tiles (8-chains) so
    the staged tiles' extra vector adds spread out.
  - Elementwise: scalar does sigmoid/tanh straight from PSUM; the
    c_t/h chain is split across VectorE and GpSimd; c_prev re-loads as
    fp16 [batch, H] tiles (error ~2e-3 absolute in c_t, well inside
    the tolerance).  The final tile pipelines its i-path in 256-col
    chunks so tanh(c_t) is ready before the o-chain ends; only
    sigmoid(o) + multiply + one store trail the last matmul (~2 us).
"""

import sys

if "/opt/trn_rl_repo" not in sys.path:
    sys.path.insert(0, "/opt/trn_rl_repo")

import numpy as np

import concourse.bacc as bacc
import concourse.mybir as mybir
import concourse.tile as tile
from concourse import bass_utils

N_CORES = 8
B, D, H = 16384, 512, 512
BL = B // N_CORES          # 2048 rows per core
G4 = 4 * H                 # 2048, the concatenated gate dim
KT = D // 128              # 4 k-tiles
BT = BL // 128             # 16 batch tiles per core
WARMUP_MMS = 8             # junk matmuls to lift the HAM clock gate
N_STAGED = 4               # batch tiles that run X@W-only while C/U load
BANK_ORDER = (3, 1, 0, 2)  # c_hat, i, f, o — deep chain first
LAST_ORDER = (3, 0, 1, 2)  # final tile: f early so c_t closes pre-end
# staged tiles interleaved with fused ones to spread their vector adds
TILE_ORDER = (0, 4, 1, 5, 2, 6, 3, 7, 8, 9, 10, 11, 12, 13, 14, 15)
# cn (c_prev fp16, natural layout) four-tile blocks follow TILE_ORDER
CN_BLOCKS = ((0, 4, 1, 5), (2, 6, 3, 7), (8, 9, 10, 11), (12, 13, 14, 15))

_cached = {}


def _build(has_bias: bool):
    key = has_bias
    if key in _cached:
        return _cached[key]

    f32 = mybir.dt.float32
    f16 = mybir.dt.float16
    AF = mybir.ActivationFunctionType
    Alu = mybir.AluOpType

    nc = bacc.Bacc("TRN2", target_bir_lowering=False, debug=False,
                   enable_asserts=False, enable_partition_id=False)
    # X^T / C^T per (k, batch-half): [128, 1024] fp16
    xa = [nc.dram_tensor(f"xa{k}", [128, 1024], f16, kind="ExternalInput")
          for k in range(KT)]
    xb = [nc.dram_tensor(f"xb{k}", [128, 1024], f16, kind="ExternalInput")
          for k in range(KT)]
    ca = [nc.dram_tensor(f"ca{k}", [128, 1024], f16, kind="ExternalInput")
          for k in range(KT)]
    cb = [nc.dram_tensor(f"cb{k}", [128, 1024], f16, kind="ExternalInput")
          for k in range(KT)]
    # W / U per gate: [128, 4*512] fp16 (the 4 k-blocks along columns)
    wd = [nc.dram_tensor(f"w{jc}", [128, 4 * 512], f16, kind="ExternalInput")
          for jc in range(4)]
    ud = [nc.dram_tensor(f"u{jc}", [128, 4 * 512], f16, kind="ExternalInput")
          for jc in range(4)]
    # c_prev natural-layout fp16, 4 tiles per block in TILE_ORDER
    cnb_d = [nc.dram_tensor(f"cnb{g}", [128, 4 * H], f16,
                            kind="ExternalInput")
             for g in range(4)]
    if has_bias:
        bias_bc = nc.dram_tensor("bias_bc", [128, G4], f32,
                                 kind="ExternalInput")
    h_out = nc.dram_tensor("h_out", [BL, H], f32, kind="ExternalOutput")

    with tile.TileContext(nc) as tc:
        with (
            tc.tile_pool(name="const", bufs=1) as const,
            tc.tile_pool(name="px", bufs=1) as px_p,
            tc.tile_pool(name="psum", bufs=8, space="PSUM") as psum,
            tc.tile_pool(name="gates", bufs=8) as gate_p,
            tc.tile_pool(name="tmp1", bufs=4) as tmp1_p,
            tc.tile_pool(name="tmp2", bufs=4) as tmp2_p,
            tc.tile_pool(name="hsb", bufs=3) as h_p,
        ):
            # PE warm-up: burn the boot->operand window on junk matmuls
            # so the HAM clock gate sees sustained activity.  gpsimd
            # memset: its sequencer preamble finishes earliest, so the
            # first junk matmul issues ~2 us sooner than via vector.
            junk = const.tile([128, 512], f16, tag="junk", name="junk")
            nc.gpsimd.memset(junk[:], 0.0)
            # Pre-create the PSUM tiles of the first 8 phase-A chains
            # and aim the junk matmuls at them: real chains begin
            # start=True (bank reset), and targeting later-read tiles
            # keeps bacc's DCE from dropping the junk.
            psA = {}
            for jc in (3, 2):
                for bt in range(N_STAGED):
                    psA[(jc, bt)] = psum.tile([128, 512], f32, tag="ps",
                                              name=f"psA{jc}_{bt}")
            warm_tgts = list(psA.values())
            for i in range(WARMUP_MMS):
                nc.tensor.matmul(
                    warm_tgts[i % 8][:], junk[:, 0:128], junk[:],
                    start=True, stop=True,
                )

            # SBUF operand tiles
            def sb(name, cols, dt=f16):
                return const.tile([128, cols], dt, tag=name, name=name)

            XA = [sb(f"XA{k}", 1024) for k in range(KT)]
            XB = [sb(f"XB{k}", 1024) for k in range(KT)]
            CA = [sb(f"CA{k}", 1024) for k in range(KT)]
            CB = [sb(f"CB{k}", 1024) for k in range(KT)]
            Wg = [sb(f"W{jc}", 4 * 512) for jc in range(4)]
            Ug = [sb(f"U{jc}", 4 * 512) for jc in range(4)]

            def Wt(jc, k):
                return Wg[jc][:, k * 512:(k + 1) * 512]

            def Ut(jc, k):
                return Ug[jc][:, k * 512:(k + 1) * 512]
            CNB = [sb(f"CNB{g}", 4 * H) for g in range(4)]

            # Four serial DMA lanes.  W and X/C lanes start immediately
            # and split HBM two ways so the first chains' operands land
            # fastest; the U lane gates on the last W piece and the cn
            # lane on ca3, so late data never steals early bandwidth.
            # Within a lane each item gates on the item TWO back
            # (2 transfers in flight hide semaphore latency).
            laneW = [(Wg[jc], wd[jc]) for jc in (3, 2, 1, 0)]
            laneU = [(Ug[jc], ud[jc]) for jc in (3, 1, 0, 2)]
            laneX = ([(XA[k], xa[k]) for k in range(KT)]
                     + [(CA[k], ca[k]) for k in range(KT)]
                     + [(XB[k], xb[k]) for k in range(KT)]
                     + [(CB[k], cb[k]) for k in range(KT)])
            laneCN = [(CNB[g], cnb_d[g]) for g in range(4)]
            lane_gate = {id(laneU): laneW[-1][0], id(laneCN): CA[KT - 1]}
            for li, lane in enumerate((laneW, laneX, laneU, laneCN)):
                start_gate = lane_gate.get(id(lane))
                for wave, (dst, dsrc) in enumerate(lane):
                    gate_tile = None
                    if wave > 1:
                        gate_tile = lane[wave - 2][0]
                    elif start_gate is not None:
                        gate_tile = start_gate
                    if gate_tile is not None:
                        nc.vector.tensor_copy(dst[0:1, 0:1],
                                              gate_tile[0:1, 0:1])
                        eng = nc.sync
                    else:
                        eng = nc.scalar if li else nc.sync
                    eng.dma_start(out=dst[:], in_=dsrc.ap())
            if has_bias:
                bias_sb = const.tile([128, G4], f32, tag="bias")
                nc.sync.dma_start(out=bias_sb[:], in_=bias_bc.ap())

            def lhs(k, bt):
                q, r = divmod(bt, 8)
                T = (XA, XB)[q]
                return T[k][:, r * 128:(r + 1) * 128]

            def lhc(k, bt):
                q, r = divmod(bt, 8)
                T = (CA, CB)[q]
                return T[k][:, r * 128:(r + 1) * 128]

            # Phase A: X@W for staged tiles, gate-major sequential
            # 4-chains (matches W arrival: all jc=3 pieces, then jc=2,
            # ...), staged to SBUF fp16.
            PX = [px_p.tile([128, G4], f16, tag=f"px{bt}", name=f"px{bt}")
                  for bt in range(N_STAGED)]
            for jc in (3, 2, 1, 0):
                for bt in range(N_STAGED):
                    if (jc, bt) in psA:
                        ps = psA[(jc, bt)]
                    else:
                        ps = psum.tile([128, 512], f32, tag="ps",
                                       name=f"psA{jc}_{bt}")
                    for k in range(KT):
                        nc.tensor.matmul(
                            ps[:], lhs(k, bt), Wt(jc, k),
                            start=(k == 0), stop=(k == KT - 1),
                        )
                    nc.scalar.activation(
                        PX[bt][:, jc * 512:(jc + 1) * 512], ps[:], AF.Copy
                    )

            def cn_ap(bt):
                for g, blk in enumerate(CN_BLOCKS):
                    if bt in blk:
                        j = blk.index(bt)
                        return CNB[g][:, j * H:(j + 1) * H]
                raise AssertionError

            # Phase B: per tile, fill the four gate banks (sequential
            # chains) and run the eviction/elementwise chain.
            for bt in TILE_ORDER:
                bsl = slice(bt * 128, (bt + 1) * 128)
                last = bt == BT - 1
                cn = cn_ap(bt)
                staged = bt < N_STAGED
                order = LAST_ORDER if last else BANK_ORDER
                gates = {}
                for jc in order:
                    jsl = slice(jc * 512, (jc + 1) * 512)
                    ps = psum.tile([128, 512], f32, tag="ps",
                                   name=f"psB{bt}_{jc}")
                    if staged:
                        for k in range(KT):
                            nc.tensor.matmul(
                                ps[:], lhc(k, bt), Ut(jc, k),
                                start=(k == 0), stop=(k == KT - 1),
                            )
                        nc.vector.tensor_tensor(
                            ps[:], ps[:], PX[bt][:, jsl], Alu.add
                        )
                    else:
                        for k in range(KT):
                            nc.tensor.matmul(
                                ps[:], lhs(k, bt), Wt(jc, k),
                                start=(k == 0), stop=False,
                            )
                        for k in range(KT):
                            nc.tensor.matmul(
                                ps[:], lhc(k, bt), Ut(jc, k),
                                start=False, stop=(k == KT - 1),
                            )
                    if has_bias:
                        nc.vector.tensor_tensor(
                            ps[:], ps[:], bias_sb[:, jsl], Alu.add
                        )
                    g = gate_p.tile([128, 512], f32, tag="g",
                                    name=f"g{bt}_{jc}")
                    if last and jc in (1, 2):
                        # final tile: halve the eviction latency of the
                        # i and o gates so the closing chain pipelines.
                        for hs in range(2):
                            csl = slice(hs * 256, (hs + 1) * 256)
                            nc.scalar.activation(
                                g[:, csl], ps[:, csl], AF.Sigmoid
                            )
                    else:
                        nc.scalar.activation(
                            g[:], ps[:], AF.Tanh if jc == 3 else AF.Sigmoid
                        )
                    gates[jc] = g
                    # Chain steps as their inputs become ready.
                    if jc == 0:
                        t1 = tmp1_p.tile([128, H], f32, tag="t1",
                                         name=f"t1_{bt}")
                        nc.gpsimd.tensor_tensor(
                            t1[:], gates[0][:], cn, Alu.mult
                        )
                    elif jc == 1:
                        t2 = tmp2_p.tile([128, H], f32, tag="t2",
                                         name=f"t2_{bt}")
                        if last:
                            # 256-col chunks so tanh(c_t) lands before
                            # the o-chain finishes.
                            tct = tmp2_p.tile([128, H], f32, tag="tct",
                                              name=f"tct{bt}")
                            for hs in range(2):
                                csl = slice(hs * 256, (hs + 1) * 256)
                                nc.vector.tensor_tensor(
                                    t2[:, csl], gates[1][:, csl],
                                    gates[3][:, csl], Alu.mult,
                                )
                                nc.vector.tensor_tensor(
                                    t1[:, csl], t1[:, csl], t2[:, csl],
                                    Alu.add,
                                )
                                nc.scalar.activation(
                                    tct[:, csl], t1[:, csl], AF.Tanh
                                )
                        else:
                            nc.vector.tensor_tensor(
                                t2[:], gates[1][:], gates[3][:], Alu.mult
                            )
                    elif jc == 2:
                        if not last:
                            nc.vector.tensor_tensor(t1[:], t1[:], t2[:],
                                                    Alu.add)
                            tct = tmp2_p.tile([128, H], f32, tag="tct",
                                              name=f"tct{bt}")
                            nc.scalar.activation(tct[:], t1[:], AF.Tanh)
                        hsb = h_p.tile([128, H], f32, tag="h", name=f"h{bt}")
                        if last:
                            for hs in range(2):
                                csl = slice(hs * 256, (hs + 1) * 256)
                                nc.vector.tensor_tensor(
                                    hsb[:, csl], gates[2][:, csl],
                                    tct[:, csl], Alu.mult,
                                )
                            nc.sync.dma_start(out=h_out.ap()[bsl, :],
                                              in_=hsb[:])
                        else:
                            nc.gpsimd.tensor_tensor(
                                hsb[:], gates[2][:], tct[:], Alu.mult
                            )
                            nc.sync.dma_start(out=h_out.ap()[bsl, :],
                                              in_=hsb[:])

    nc.compile()
    _cached[key] = nc
    return nc


def _prep(X_t, c_prev, W_f, W_i, W_o, W_c, U_f, U_i, U_o, U_c, b_f, b_i, b_o, b_c):
    """Host-side (free) preprocessing: concat, cast, transpose, shard."""
    f16 = np.float16
    W = np.concatenate([W_f, W_i, W_o, W_c], axis=1).astype(f16)
    U = np.concatenate([U_f, U_i, U_o, U_c], axis=1).astype(f16)
    b = np.concatenate([b_f, b_i, b_o, b_c], axis=0).astype(np.float32)
    has_bias = bool(np.any(b != 0.0))

    X16 = np.asarray(X_t).astype(f16)
    C16 = np.asarray(c_prev).astype(f16)

    wp = {}
    for jc in range(4):
        wp[f"w{jc}"] = np.ascontiguousarray(np.concatenate(
            [W[k * 128:(k + 1) * 128, jc * 512:(jc + 1) * 512]
             for k in range(KT)], axis=1))
        wp[f"u{jc}"] = np.ascontiguousarray(np.concatenate(
            [U[k * 128:(k + 1) * 128, jc * 512:(jc + 1) * 512]
             for k in range(KT)], axis=1))

    in_maps = []
    for i in range(N_CORES):
        sl = slice(i * BL, (i + 1) * BL)
        XT = X16[sl].T  # [D, BL]
        CT = C16[sl].T
        Cn = C16[sl]    # [BL, D] natural
        m = dict(wp)
        for k in range(KT):
            ks = slice(k * 128, (k + 1) * 128)
            m[f"xa{k}"] = np.ascontiguousarray(XT[ks, 0:1024])
            m[f"xb{k}"] = np.ascontiguousarray(XT[ks, 1024:2048])
            m[f"ca{k}"] = np.ascontiguousarray(CT[ks, 0:1024])
            m[f"cb{k}"] = np.ascontiguousarray(CT[ks, 1024:2048])
        for g, blk in enumerate(CN_BLOCKS):
            m[f"cnb{g}"] = np.ascontiguousarray(
                np.concatenate([Cn[bt * 128:(bt + 1) * 128, :] for bt in blk],
                               axis=1)
            )
        if has_bias:
            m["bias_bc"] = np.ascontiguousarray(
                np.broadcast_to(b[None, :], (128, G4))
            )
        in_maps.append(m)
    return in_maps, has_bias


def kernel(**inputs):
    in_maps, has_bias = _prep(**inputs)
    nc = _build(has_bias)
    last_err = None
    for _ in range(3):
        try:
            res = bass_utils.run_bass_kernel_spmd(
                nc, in_maps, core_ids=list(range(N_CORES))
            )
            break
        except Exception as e:  # intermittent device wedge: retry
            last_err = e
            import time
            time.sleep(5)
    else:
        raise last_err
    return np.concatenate([res.results[i]["h_out"] for i in range(N_CORES)],
                          axis=0)
